# revision 22
# baseline (speedup 1.0000x reference)
"""Binary 3-layer CNN (sign activations + sign weights) on 8 NeuronCores.

Strategy: pure data parallel — 64 images -> 8 cores x 8 images, fp8 compute.
All matmul operands are exactly +-1/0 -> fp8e4m3 with fp32 PSUM accumulation
is numerically exact.

Per core, 2 batches of 4 images; SBUF partition layout [128 = (4 img, 32 ch)].
The three convs are FUSED per strip of R output rows with halo recompute
(strip s computes R+4 rows of h0 -> R+2 rows of h1 -> R output rows), so h0
and h1 never leave SBUF; only the padded sign(x) plane stages through DRAM.

 - conv0 (1->32ch): all 9 taps packed into K: input replicated into 36
   partitions (tap, img) via 9 shifted DMAs from the extended-pad sign(x)
   plane; one matmul per output row-pair (K=36, M=128, N=512).
 - conv1 (32->32ch): fp8 DoubleRow matmuls, 2 taps per pass: K=128 partitions
   x 2 k-subtiles; the rhs k-subtile offset is a free-dim shift on the padded
   input tile (4D AP), so 9 taps cost 5 passes instead of 9. Tap pairs share
   one (row,col) shift delta; the odd 9th tap is paired with zero weights.
 - conv2 (32->1ch): same DoubleRow pairing, M=4 (one column per image),
   psum [4, 8, 256] -> bf16 output (conv2 sums are even integers <= 288,
   exact in bf16).
PSUM is used as [128, 8, 256] 4-bank supertiles (4 row-pair matmul groups +
one batched eviction). sign() eviction: psum -> fp8 via ScalarE Sign (conv0)
and VectorE clamp(-1,1) (conv1; sums are integers so clamp == sign).
Issue order is software-pipelined (A of strip s+1 before B of strip s) to
hide eviction tails from the in-order PE queue.
"""

import numpy as np
import ml_dtypes

import concourse.bass as bass
import concourse.mybir as mybir
import concourse.tile as tile
from concourse import bacc
from concourse.bass_utils import run_bass_kernel_spmd

FP8 = mybir.dt.float8e4
BF16 = mybir.dt.bfloat16
F32 = mybir.dt.float32
AF = mybir.ActivationFunctionType
ALU = mybir.AluOpType
DRM = mybir.MatmulPerfMode.DoubleRow

N_CORES = 8
IMG_PER_CORE = 8
B = 4          # images per partition-batch
H = W = 256
WP = 258       # padded width (1 col pad each side)
HE = 262       # extended padded height: row = x row + 3
R = 64         # strip rows (output rows per strip)
NS = H // R    # strips per batch
NB = IMG_PER_CORE // B  # batches per core

# DoubleRow tap pairs: both taps of a pair share one flat shift delta
# (dy*WP + dx); the 9th tap is paired with zero weights (k-slot 1 unused).
PAIRS = [
    ((0, 0), (0, 1)),
    ((1, 0), (1, 1)),
    ((2, 0), (2, 1)),
    ((0, 2), (1, 2)),
    ((2, 2), None),
]


def _dr_rhs(hin, r, pair):
    """4D DoubleRow rhs AP: [128, ksub=2, rows=2, cols=256] with the ksub
    dim stepping by the tap-pair's shift delta over the padded tile."""
    (dy0, dx0), t1 = pair
    # the zero-weight dummy slot points one row up: always inside the tile
    delta = -WP if t1 is None else (t1[0] - dy0) * WP + (t1[1] - dx0)
    sl = hin[:, r + dy0:r + dy0 + 2, dx0:dx0 + 256]
    return bass.AP(
        tensor=sl.tensor, offset=sl.offset,
        ap=[list(sl.ap[0]), [delta, 2], list(sl.ap[1]), list(sl.ap[2])])


def _build_program(stages=('0', 'A', 'B', 'C')):
    nc = bacc.Bacc("TRN2", target_bir_lowering=False, debug=False)

    x_in = nc.dram_tensor("x", [IMG_PER_CORE, H, W], F32, kind="ExternalInput")
    s0_in = nc.dram_tensor("s0", [36, 128], FP8, kind="ExternalInput")
    s1_in = nc.dram_tensor("s1", [128, 5, 2, 128], FP8, kind="ExternalInput")
    s2_in = nc.dram_tensor("s2", [128, 5, 2, 16], FP8, kind="ExternalInput")
    out_d = nc.dram_tensor("out", [IMG_PER_CORE, H, W], BF16,
                           kind="ExternalOutput")

    # extended-pad sign(x), 3 column-shifted copies (one per conv dx tap):
    # xs3[b, dx, g, r, c] = sign(x)[img b*B+g, row r-3, col c+dx-1] with
    # zero padding outside; row = x row + 3 (rows 0-2 and 259-261 zero)
    xs3_d = nc.dram_tensor("xs3", [NB, 3, B, HE, 256], FP8)

    with tile.TileContext(nc) as tc:
        with (
            tc.tile_pool(name="const", bufs=1) as cpool,
            tc.tile_pool(name="xprep", bufs=1) as xpool,
            tc.tile_pool(name="xrep", bufs=2) as xrpool,
            tc.tile_pool(name="h0", bufs=2) as h0pool,
            tc.tile_pool(name="h1", bufs=2) as h1pool,
            tc.tile_pool(name="cout", bufs=2) as cpool2,
            tc.tile_pool(name="psum", bufs=4, space="PSUM") as pspool,
        ):
            # --- constants: stationary weights + a zero tile ---
            s0t = cpool.tile([36, 128], FP8, tag="s0")
            nc.sync.dma_start(out=s0t[:, :], in_=s0_in[:, :])
            s1t = cpool.tile([128, 5, 2, 128], FP8, tag="s1")
            nc.sync.dma_start(out=s1t[:, :, :, :], in_=s1_in[:, :, :, :])
            s2t = cpool.tile([128, 5, 2, 16], FP8, tag="s2")
            nc.sync.dma_start(out=s2t[:, :, :, :], in_=s2_in[:, :, :, :])
            zt = cpool.tile([128, 3 * 256], FP8, tag="zt")
            nc.gpsimd.memset(zt[:, :], 0.0)

            # --- pre-zero xs3 pad rows (cols baked into the stores) ---
            for b in range(NB):
                for r0, r1 in ((0, 3), (HE - 3, HE)):
                    nc.scalar.dma_start(
                        out=xs3_d[b, :, :, r0:r1, :].rearrange(
                            "a g r c -> (a g) r c"),
                        in_=zt[0:12, :].rearrange("p (r c) -> p r c", r=3))

            def stage_0(b):
                """sign(x) -> extended-pad fp8 planes in DRAM, batch b.
                One load + one sign for all 4 images (x rows in partitions,
                (img, 128-row block) merged in the free dim), then 12 shifted
                stores spread over the three DMA queues."""
                xf = xpool.tile([128, 2 * B, W], F32, tag="xf")
                xp = xpool.tile([128, 2 * B, WP], FP8, tag="xp")
                nc.gpsimd.memset(xp[:, :, 0:1], 0.0)
                nc.gpsimd.memset(xp[:, :, WP - 1:WP], 0.0)
                queues = [nc.gpsimd, nc.scalar, nc.sync]
                for h, q in ((0, nc.scalar), (1, nc.sync)):
                    q.dma_start(
                        out=xf[:, B * h:B * (h + 1), :],
                        in_=bass.AP(tensor=x_in,
                                    offset=(b * 2 + h) * (B // 2) * H * W,
                                    ap=[[W, 128], [128 * W, B], [1, W]]))
                    nc.scalar.activation(xp[:, B * h:B * (h + 1), 1:W + 1],
                                         xf[:, B * h:B * (h + 1), :], AF.Sign)
                    for g in range(B // 2 * h, B // 2 * (h + 1)):
                        for dx in range(3):
                            queues[(g * 3 + dx) % 3].dma_start(
                                out=bass.AP(
                                    tensor=xs3_d,
                                    offset=(((b * 3 + dx) * B + g) * HE + 3)
                                    * 256,
                                    ap=[[256, 128], [128 * 256, 2], [1, 256]]),
                                in_=xp[:, 2 * g:2 * g + 2, dx:dx + 256])

            if '0' in stages:
                stage_0(0)

            if True:
                ht0s, ht1s = {}, {}

                def stage_a_units(s, b):
                    """conv0 strip s: h0 rows [sR-2, sR+R+2) -> ht0 tile
                    (tile row i = h0 row sR-2+i). Returns (ht0, units)."""
                    xt = xrpool.tile([36, R + 4, 256], FP8, tag="xrep",
                                     name="xt")
                    for dy, q in ((0, nc.sync), (1, nc.scalar),
                                  (2, nc.gpsimd)):
                        q.dma_start(
                            out=xt[12 * dy:12 * dy + 12, :, :],
                            in_=xs3_d[b, :, :,
                                      s * R + dy:s * R + dy + R + 4, :]
                            .rearrange("a g r c -> (a g) r c"))
                    ht0 = h0pool.tile([128, R + 4, WP], FP8, tag="h0",
                                      name="ht0")
                    nc.gpsimd.memset(ht0[:, :, 0:1], 0.0)
                    nc.gpsimd.memset(ht0[:, :, WP - 1:WP], 0.0)

                    def unit(r0, last):
                        nrow = min(4, R + 4 - r0)
                        ps = pspool.tile([128, 4, 256], F32, tag="ps",
                                         name="psA")
                        for q in range(nrow // 2):
                            nc.tensor.matmul(
                                ps[:, 2 * q:2 * q + 2, :], s0t[:, :],
                                xt[:, r0 + 2 * q:r0 + 2 * q + 2, :],
                                start=True, stop=True)
                        if (r0 // 4) % 2 == 0:
                            nc.scalar.activation(
                                ht0[:, r0:r0 + nrow, 1:W + 1],
                                ps[:, 0:nrow, :], AF.Sign)
                        else:
                            nc.vector.tensor_scalar(
                                ht0[:, r0:r0 + nrow, 1:W + 1],
                                ps[:, 0:nrow, :], -1.0, 1.0,
                                ALU.max, ALU.min)
                        if last:
                            # boundary: h0 pad rows (-1 / 256) must be zero
                            if s == 0:
                                nc.gpsimd.memset(ht0[:, 1:2, :], 0.0)
                            if s == NS - 1:
                                nc.gpsimd.memset(ht0[:, R + 2:R + 3, :], 0.0)

                    r0s = list(range(0, R + 4, 4))
                    return ht0, [
                        (lambda r0=r0, last=(r0 == r0s[-1]): unit(r0, last))
                        for r0 in r0s]

                def stage_b_units(s, ht0):
                    """conv1 strip s: h1 rows [sR-1, sR+R+1) -> ht1 tile
                    (tile row i = h1 row sR-1+i); input ht0."""
                    ht1 = h1pool.tile([128, R + 2, WP], FP8, tag="h1",
                                      name="ht1")
                    nc.gpsimd.memset(ht1[:, :, 0:1], 0.0)
                    nc.gpsimd.memset(ht1[:, :, WP - 1:WP], 0.0)

                    def unit(r0, last):
                        nrow = min(4, R + 2 - r0)
                        ps = pspool.tile([128, 4, 256], F32, tag="ps",
                                         name="psB")
                        for q in range(nrow // 2):
                            for t, pair in enumerate(PAIRS):
                                nc.tensor.matmul(
                                    ps[:, 2 * q:2 * q + 2, :], s1t[:, t, :, :],
                                    _dr_rhs(ht0, r0 + 2 * q, pair),
                                    start=(t == 0), stop=(t == 4),
                                    perf_mode=DRM)
                        # alternate eviction engines to keep the psum
                        # rotation fed (clamp(-1,1) == sign for int sums)
                        if (r0 // 4) % 2 == 0:
                            nc.scalar.activation(
                                ht1[:, r0:r0 + nrow, 1:W + 1],
                                ps[:, 0:nrow, :], AF.Sign)
                        else:
                            nc.vector.tensor_scalar(
                                ht1[:, r0:r0 + nrow, 1:W + 1],
                                ps[:, 0:nrow, :], -1.0, 1.0,
                                ALU.max, ALU.min)
                        if last:
                            # boundary: h1 pad rows (-1 / 256) must be zero
                            if s == 0:
                                nc.gpsimd.memset(ht1[:, 0:1, :], 0.0)
                            if s == NS - 1:
                                nc.gpsimd.memset(ht1[:, R + 1:R + 2, :], 0.0)

                    r0s = list(range(0, R + 2, 4))
                    return ht1, [
                        (lambda r0=r0, last=(r0 == r0s[-1]): unit(r0, last))
                        for r0 in r0s]

                def stage_c_units(s, ht1, b):
                    """conv2 strip s: out rows [sR, sR+R); input ht1."""
                    ot = cpool2.tile([B, R, W], BF16, tag="c_out", name="ot")

                    def unit(r0, last):
                        ps = pspool.tile([128, 4, 256], F32, tag="ps",
                                         name="psC")
                        for q in range(2):
                            for t, pair in enumerate(PAIRS):
                                nc.tensor.matmul(
                                    ps[0:B, 2 * q:2 * q + 2, :],
                                    s2t[:, t, :, 0:B],
                                    _dr_rhs(ht1, r0 + 2 * q, pair),
                                    start=(t == 0), stop=(t == 4),
                                    perf_mode=DRM)
                        if (r0 // 4) % 2 == 0:
                            nc.vector.tensor_copy(
                                ot[:, r0:r0 + 4, :], ps[0:B, :, :])
                        else:
                            nc.scalar.activation(
                                ot[:, r0:r0 + 4, :], ps[0:B, :, :], AF.Copy)
                        if last:
                            nc.gpsimd.dma_start(
                                out=out_d[b * B:(b + 1) * B,
                                          s * R:s * R + R, :],
                                in_=ot[:, :, :])

                    r0s = list(range(0, R, 4))
                    return [
                        (lambda r0=r0, last=(r0 == r0s[-1]): unit(r0, last))
                        for r0 in r0s]

                # software pipeline, depth 2, interleaved at supertile
                # granularity and run GLOBALLY over both batches: fast-matmul
                # A units ride alongside slow-matmul B/C units so the
                # in-order PE never drains the 4-buffer PSUM rotation
                # waiting on an eviction.
                strips = [(b, s) for b in range(NB) for s in range(NS)]
                for gi in range(len(strips) + 2):
                    units = []
                    if gi < len(strips) and 'A' in stages:
                        gb, gs = strips[gi]
                        # next batch's input prep ahead of its first strip
                        if '0' in stages and gs == 0 and gb + 1 < NB:
                            stage_0(gb + 1)
                        ht0s[gi], ua = stage_a_units(gs, gb)
                        units.append(ua)
                    if 0 <= gi - 1 < len(strips) and 'B' in stages \
                            and gi - 1 in ht0s:
                        gb, gs = strips[gi - 1]
                        ht1s[gi - 1], ub = stage_b_units(gs,
                                                         ht0s.pop(gi - 1))
                        units.append(ub)
                    if 0 <= gi - 2 < len(strips) and 'C' in stages \
                            and gi - 2 in ht1s:
                        gb, gs = strips[gi - 2]
                        units.append(stage_c_units(gs, ht1s.pop(gi - 2), gb))
                    for i in range(max(map(len, units), default=0)):
                        for u in units:
                            if i < len(u):
                                u[i]()
    nc.compile()
    return nc


def _host_weights(w0, w1, w2):
    """Pack sign(w) into fp8 stationary matrices (see module docstring)."""
    f8 = ml_dtypes.float8_e4m3
    sg = lambda w: np.sign(np.asarray(w, np.float32))
    w0s, w1s, w2s = sg(w0), sg(w1), sg(w2)  # [32,1,3,3],[32,32,3,3],[1,32,3,3]
    s0 = np.zeros((36, 128), np.float32)
    s1 = np.zeros((128, 5, 2, 128), np.float32)
    s2 = np.zeros((128, 5, 2, 16), np.float32)
    for g in range(B):
        for dy in range(3):
            for dx in range(3):
                s0[(dy * 3 + dx) * 4 + g, g * 32:(g + 1) * 32] = \
                    w0s[:, 0, dy, dx]
        for t, ((dy0, dx0), t1) in enumerate(PAIRS):
            s1[g * 32:(g + 1) * 32, t, 0, g * 32:(g + 1) * 32] = \
                w1s[:, :, dy0, dx0].T  # [ci, co]
            s2[g * 32:(g + 1) * 32, t, 0, g] = w2s[0, :, dy0, dx0]
            if t1 is not None:
                s1[g * 32:(g + 1) * 32, t, 1, g * 32:(g + 1) * 32] = \
                    w1s[:, :, t1[0], t1[1]].T
                s2[g * 32:(g + 1) * 32, t, 1, g] = w2s[0, :, t1[0], t1[1]]
    return s0.astype(f8), s1.astype(f8), s2.astype(f8)


_NC_CACHE = {}


def kernel(x, w0, w1, w2):
    if "nc" not in _NC_CACHE:
        _NC_CACHE["nc"] = _build_program()
    nc = _NC_CACHE["nc"]
    s0, s1, s2 = _host_weights(w0, w1, w2)
    x = np.asarray(x, np.float32).reshape(64, H, W)
    in_maps = [
        {"x": np.ascontiguousarray(x[i * IMG_PER_CORE:(i + 1) * IMG_PER_CORE]),
         "s0": s0, "s1": s1, "s2": s2}
        for i in range(N_CORES)
    ]
    res = run_bass_kernel_spmd(nc, in_maps, list(range(N_CORES)))
    out = np.stack([np.asarray(res.results[i]["out"], np.float32)
                    for i in range(N_CORES)])
    return out.reshape(64, 1, H, W)


# revision 23
# speedup vs baseline: 1.1245x; 1.1245x over previous
"""Binary 3-layer CNN (sign activations + sign weights) on 8 NeuronCores.

Strategy: pure data parallel — 64 images -> 8 cores x 8 images, fp8 compute.
All matmul operands are exactly +-1/0 -> fp8e4m3 with fp32 PSUM accumulation
is numerically exact.

Per core, 2 batches of 4 images; SBUF partition layout [128 = (4 img, 32 ch)].
The three convs are FUSED per strip of R output rows with halo recompute
(strip s computes R+4 rows of h0 -> R+2 rows of h1 -> R output rows), so h0
and h1 never leave SBUF; only the padded sign(x) plane stages through DRAM.

 - conv0 (1->32ch): all 9 taps packed into K: input replicated into 36
   partitions (tap, img) via 9 shifted DMAs from the extended-pad sign(x)
   plane; one matmul per output row-pair (K=36, M=128, N=512).
 - conv1 (32->32ch): fp8 DoubleRow matmuls, 2 taps per pass: K=128 partitions
   x 2 k-subtiles; the rhs k-subtile offset is a free-dim shift on the padded
   input tile (4D AP), so 9 taps cost 5 passes instead of 9. Tap pairs share
   one (row,col) shift delta; the odd 9th tap is paired with zero weights.
 - conv2 (32->1ch): same DoubleRow pairing, M=4 (one column per image),
   psum [4, 8, 256] -> bf16 output (conv2 sums are even integers <= 288,
   exact in bf16).
PSUM is used as [128, 8, 256] 4-bank supertiles (4 row-pair matmul groups +
one batched eviction). sign() eviction: psum -> fp8 via ScalarE Sign (conv0)
and VectorE clamp(-1,1) (conv1; sums are integers so clamp == sign).
Issue order is software-pipelined (A of strip s+1 before B of strip s) to
hide eviction tails from the in-order PE queue.
"""

import numpy as np
import ml_dtypes

import concourse.bass as bass
import concourse.mybir as mybir
import concourse.tile as tile
from concourse import bacc
from concourse.bass_utils import run_bass_kernel_spmd

FP8 = mybir.dt.float8e4
BF16 = mybir.dt.bfloat16
F32 = mybir.dt.float32
AF = mybir.ActivationFunctionType
ALU = mybir.AluOpType
DRM = mybir.MatmulPerfMode.DoubleRow

N_CORES = 8
IMG_PER_CORE = 8
B = 4          # images per partition-batch
H = W = 256
WP = 258       # padded width (1 col pad each side)
HE = 262       # extended padded height: row = x row + 3
R = 64         # strip rows (output rows per strip)
NS = H // R    # strips per batch
NB = IMG_PER_CORE // B  # batches per core

# DoubleRow tap pairs: both taps of a pair share one flat shift delta
# (dy*WP + dx); the 9th tap is paired with zero weights (k-slot 1 unused).
PAIRS = [
    ((0, 0), (0, 1)),
    ((1, 0), (1, 1)),
    ((2, 0), (2, 1)),
    ((0, 2), (1, 2)),
    ((2, 2), None),
]


def _dr_rhs(hin, r, pair):
    """4D DoubleRow rhs AP: [128, ksub=2, rows=2, cols=256] with the ksub
    dim stepping by the tap-pair's shift delta over the padded tile."""
    (dy0, dx0), t1 = pair
    # the zero-weight dummy slot points one row up: always inside the tile
    delta = -WP if t1 is None else (t1[0] - dy0) * WP + (t1[1] - dx0)
    sl = hin[:, r + dy0:r + dy0 + 2, dx0:dx0 + 256]
    return bass.AP(
        tensor=sl.tensor, offset=sl.offset,
        ap=[list(sl.ap[0]), [delta, 2], list(sl.ap[1]), list(sl.ap[2])])


def _build_program(stages=('0', 'A', 'B', 'C')):
    nc = bacc.Bacc("TRN2", target_bir_lowering=False, debug=False)

    x_in = nc.dram_tensor("x", [IMG_PER_CORE, H, W], F32, kind="ExternalInput")
    s0_in = nc.dram_tensor("s0", [36, 128], FP8, kind="ExternalInput")
    s1_in = nc.dram_tensor("s1", [128, 5, 2, 128], FP8, kind="ExternalInput")
    s2_in = nc.dram_tensor("s2", [128, 5, 2, 16], FP8, kind="ExternalInput")
    out_d = nc.dram_tensor("out", [IMG_PER_CORE, H, W], BF16,
                           kind="ExternalOutput")

    # extended-pad sign(x), 3 column-shifted copies (one per conv dx tap):
    # xs3[b, dx, g, r, c] = sign(x)[img b*B+g, row r-3, col c+dx-1] with
    # zero padding outside; row = x row + 3 (rows 0-2 and 259-261 zero)
    xs3_d = nc.dram_tensor("xs3", [NB, 3, B, HE, 256], FP8)

    with tile.TileContext(nc) as tc:
        with (
            tc.tile_pool(name="const", bufs=1) as cpool,
            tc.tile_pool(name="xprep", bufs=4) as xpool,
            tc.tile_pool(name="xrep", bufs=2) as xrpool,
            tc.tile_pool(name="h0", bufs=2) as h0pool,
            tc.tile_pool(name="h1", bufs=2) as h1pool,
            tc.tile_pool(name="cout", bufs=2) as cpool2,
            tc.tile_pool(name="psum", bufs=4, space="PSUM") as pspool,
        ):
            # --- constants: stationary weights + a zero tile ---
            s0t = cpool.tile([36, 128], FP8, tag="s0")
            nc.sync.dma_start(out=s0t[:, :], in_=s0_in[:, :])
            s1t = cpool.tile([128, 5, 2, 128], FP8, tag="s1")
            nc.sync.dma_start(out=s1t[:, :, :, :], in_=s1_in[:, :, :, :])
            s2t = cpool.tile([128, 5, 2, 16], FP8, tag="s2")
            nc.sync.dma_start(out=s2t[:, :, :, :], in_=s2_in[:, :, :, :])
            zt = cpool.tile([128, 3 * 256], FP8, tag="zt")
            nc.gpsimd.memset(zt[:, :], 0.0)

            # --- pre-zero xs3 pad rows (cols baked into the stores) ---
            for b in range(NB):
                for r0, r1 in ((0, 3), (HE - 3, HE)):
                    nc.scalar.dma_start(
                        out=xs3_d[b, :, :, r0:r1, :].rearrange(
                            "a g r c -> (a g) r c"),
                        in_=zt[0:12, :].rearrange("p (r c) -> p r c", r=3))

            def stage_0(b):
                """sign(x) -> extended-pad fp8 planes in DRAM, batch b.
                One load + one sign for all 4 images (x rows in partitions,
                (img, 128-row block) merged in the free dim), then 12 shifted
                stores spread over the three DMA queues."""
                for g in range(B):
                    img = b * B + g
                    xf = xpool.tile([128, 2, W], F32, tag="xf")
                    nc.scalar.dma_start(
                        out=xf[:, :, :],
                        in_=bass.AP(tensor=x_in, offset=img * H * W,
                                    ap=[[W, 128], [128 * W, 2], [1, W]]))
                    xp = xpool.tile([128, 2, WP], FP8, tag="xp")
                    nc.scalar.activation(xp[:, :, 1:W + 1], xf[:, :, :],
                                         AF.Sign)
                    nc.gpsimd.memset(xp[:, :, 0:1], 0.0)
                    nc.gpsimd.memset(xp[:, :, WP - 1:WP], 0.0)
                    for dx in range(3):
                        nc.gpsimd.dma_start(
                            out=bass.AP(
                                tensor=xs3_d,
                                offset=(((b * 3 + dx) * B + g) * HE + 3) * 256,
                                ap=[[256, 128], [128 * 256, 2], [1, 256]]),
                            in_=xp[:, :, dx:dx + 256])

            if '0' in stages:
                stage_0(0)

            if True:
                ht0s, ht1s = {}, {}

                def stage_a_units(s, b):
                    """conv0 strip s: h0 rows [sR-2, sR+R+2) -> ht0 tile
                    (tile row i = h0 row sR-2+i). Returns (ht0, units)."""
                    xt = xrpool.tile([36, R + 4, 256], FP8, tag="xrep",
                                     name="xt")
                    for dy, q in ((0, nc.sync), (1, nc.sync),
                                  (2, nc.sync)):
                        q.dma_start(
                            out=xt[12 * dy:12 * dy + 12, :, :],
                            in_=xs3_d[b, :, :,
                                      s * R + dy:s * R + dy + R + 4, :]
                            .rearrange("a g r c -> (a g) r c"))
                    ht0 = h0pool.tile([128, R + 4, WP], FP8, tag="h0",
                                      name="ht0")
                    nc.gpsimd.memset(ht0[:, :, 0:1], 0.0)
                    nc.gpsimd.memset(ht0[:, :, WP - 1:WP], 0.0)

                    def unit(r0, last):
                        nrow = min(4, R + 4 - r0)
                        ps = pspool.tile([128, 4, 256], F32, tag="ps",
                                         name="psA")
                        for q in range(nrow // 2):
                            nc.tensor.matmul(
                                ps[:, 2 * q:2 * q + 2, :], s0t[:, :],
                                xt[:, r0 + 2 * q:r0 + 2 * q + 2, :],
                                start=True, stop=True)
                        if (r0 // 4) % 2 == 0:
                            nc.scalar.activation(
                                ht0[:, r0:r0 + nrow, 1:W + 1],
                                ps[:, 0:nrow, :], AF.Sign)
                        else:
                            nc.vector.tensor_scalar(
                                ht0[:, r0:r0 + nrow, 1:W + 1],
                                ps[:, 0:nrow, :], -1.0, 1.0,
                                ALU.max, ALU.min)
                        if last:
                            # boundary: h0 pad rows (-1 / 256) must be zero
                            if s == 0:
                                nc.gpsimd.memset(ht0[:, 1:2, :], 0.0)
                            if s == NS - 1:
                                nc.gpsimd.memset(ht0[:, R + 2:R + 3, :], 0.0)

                    r0s = list(range(0, R + 4, 4))
                    return ht0, [
                        (lambda r0=r0, last=(r0 == r0s[-1]): unit(r0, last))
                        for r0 in r0s]

                def stage_b_units(s, ht0):
                    """conv1 strip s: h1 rows [sR-1, sR+R+1) -> ht1 tile
                    (tile row i = h1 row sR-1+i); input ht0."""
                    ht1 = h1pool.tile([128, R + 2, WP], FP8, tag="h1",
                                      name="ht1")
                    nc.gpsimd.memset(ht1[:, :, 0:1], 0.0)
                    nc.gpsimd.memset(ht1[:, :, WP - 1:WP], 0.0)

                    def unit(r0, last):
                        nrow = min(4, R + 2 - r0)
                        ps = pspool.tile([128, 4, 256], F32, tag="ps",
                                         name="psB")
                        for q in range(nrow // 2):
                            for t, pair in enumerate(PAIRS):
                                nc.tensor.matmul(
                                    ps[:, 2 * q:2 * q + 2, :], s1t[:, t, :, :],
                                    _dr_rhs(ht0, r0 + 2 * q, pair),
                                    start=(t == 0), stop=(t == 4),
                                    perf_mode=DRM)
                        # alternate eviction engines to keep the psum
                        # rotation fed (clamp(-1,1) == sign for int sums)
                        if (r0 // 4) % 2 == 0:
                            nc.scalar.activation(
                                ht1[:, r0:r0 + nrow, 1:W + 1],
                                ps[:, 0:nrow, :], AF.Sign)
                        else:
                            nc.vector.tensor_scalar(
                                ht1[:, r0:r0 + nrow, 1:W + 1],
                                ps[:, 0:nrow, :], -1.0, 1.0,
                                ALU.max, ALU.min)
                        if last:
                            # boundary: h1 pad rows (-1 / 256) must be zero
                            if s == 0:
                                nc.gpsimd.memset(ht1[:, 0:1, :], 0.0)
                            if s == NS - 1:
                                nc.gpsimd.memset(ht1[:, R + 1:R + 2, :], 0.0)

                    r0s = list(range(0, R + 2, 4))
                    return ht1, [
                        (lambda r0=r0, last=(r0 == r0s[-1]): unit(r0, last))
                        for r0 in r0s]

                def stage_c_units(s, ht1, b):
                    """conv2 strip s: out rows [sR, sR+R); input ht1."""
                    ot = cpool2.tile([B, R, W], BF16, tag="c_out", name="ot")

                    def unit(r0, last):
                        ps = pspool.tile([128, 4, 256], F32, tag="ps",
                                         name="psC")
                        for q in range(2):
                            for t, pair in enumerate(PAIRS):
                                nc.tensor.matmul(
                                    ps[0:B, 2 * q:2 * q + 2, :],
                                    s2t[:, t, :, 0:B],
                                    _dr_rhs(ht1, r0 + 2 * q, pair),
                                    start=(t == 0), stop=(t == 4),
                                    perf_mode=DRM)
                        if (r0 // 4) % 2 == 0:
                            nc.vector.tensor_copy(
                                ot[:, r0:r0 + 4, :], ps[0:B, :, :])
                        else:
                            nc.scalar.activation(
                                ot[:, r0:r0 + 4, :], ps[0:B, :, :], AF.Copy)
                        if last:
                            nc.gpsimd.dma_start(
                                out=out_d[b * B:(b + 1) * B,
                                          s * R:s * R + R, :],
                                in_=ot[:, :, :])

                    r0s = list(range(0, R, 4))
                    return [
                        (lambda r0=r0, last=(r0 == r0s[-1]): unit(r0, last))
                        for r0 in r0s]

                # software pipeline, depth 2, interleaved at supertile
                # granularity and run GLOBALLY over both batches: fast-matmul
                # A units ride alongside slow-matmul B/C units so the
                # in-order PE never drains the 4-buffer PSUM rotation
                # waiting on an eviction.
                strips = [(b, s) for b in range(NB) for s in range(NS)]
                for gi in range(len(strips) + 2):
                    units = []
                    if gi < len(strips) and 'A' in stages:
                        gb, gs = strips[gi]
                        # next batch's input prep ahead of its first strip
                        if '0' in stages and gs == 0 and gb + 1 < NB:
                            stage_0(gb + 1)
                        ht0s[gi], ua = stage_a_units(gs, gb)
                        units.append(ua)
                    if 0 <= gi - 1 < len(strips) and 'B' in stages \
                            and gi - 1 in ht0s:
                        gb, gs = strips[gi - 1]
                        ht1s[gi - 1], ub = stage_b_units(gs,
                                                         ht0s.pop(gi - 1))
                        units.append(ub)
                    if 0 <= gi - 2 < len(strips) and 'C' in stages \
                            and gi - 2 in ht1s:
                        gb, gs = strips[gi - 2]
                        units.append(stage_c_units(gs, ht1s.pop(gi - 2), gb))
                    for i in range(max(map(len, units), default=0)):
                        for u in units:
                            if i < len(u):
                                u[i]()
    nc.compile()
    return nc


def _host_weights(w0, w1, w2):
    """Pack sign(w) into fp8 stationary matrices (see module docstring)."""
    f8 = ml_dtypes.float8_e4m3
    sg = lambda w: np.sign(np.asarray(w, np.float32))
    w0s, w1s, w2s = sg(w0), sg(w1), sg(w2)  # [32,1,3,3],[32,32,3,3],[1,32,3,3]
    s0 = np.zeros((36, 128), np.float32)
    s1 = np.zeros((128, 5, 2, 128), np.float32)
    s2 = np.zeros((128, 5, 2, 16), np.float32)
    for g in range(B):
        for dy in range(3):
            for dx in range(3):
                s0[(dy * 3 + dx) * 4 + g, g * 32:(g + 1) * 32] = \
                    w0s[:, 0, dy, dx]
        for t, ((dy0, dx0), t1) in enumerate(PAIRS):
            s1[g * 32:(g + 1) * 32, t, 0, g * 32:(g + 1) * 32] = \
                w1s[:, :, dy0, dx0].T  # [ci, co]
            s2[g * 32:(g + 1) * 32, t, 0, g] = w2s[0, :, dy0, dx0]
            if t1 is not None:
                s1[g * 32:(g + 1) * 32, t, 1, g * 32:(g + 1) * 32] = \
                    w1s[:, :, t1[0], t1[1]].T
                s2[g * 32:(g + 1) * 32, t, 1, g] = w2s[0, :, t1[0], t1[1]]
    return s0.astype(f8), s1.astype(f8), s2.astype(f8)


_NC_CACHE = {}


def kernel(x, w0, w1, w2):
    if "nc" not in _NC_CACHE:
        _NC_CACHE["nc"] = _build_program()
    nc = _NC_CACHE["nc"]
    s0, s1, s2 = _host_weights(w0, w1, w2)
    x = np.asarray(x, np.float32).reshape(64, H, W)
    in_maps = [
        {"x": np.ascontiguousarray(x[i * IMG_PER_CORE:(i + 1) * IMG_PER_CORE]),
         "s0": s0, "s1": s1, "s2": s2}
        for i in range(N_CORES)
    ]
    res = run_bass_kernel_spmd(nc, in_maps, list(range(N_CORES)))
    out = np.stack([np.asarray(res.results[i]["out"], np.float32)
                    for i in range(N_CORES)])
    return out.reshape(64, 1, H, W)


# revision 24
# speedup vs baseline: 1.2335x; 1.0969x over previous
"""Binary 3-layer CNN (sign activations + sign weights) on 8 NeuronCores.

Strategy: pure data parallel — 64 images -> 8 cores x 8 images, fp8 compute.
All matmul operands are exactly +-1/0 -> fp8e4m3 with fp32 PSUM accumulation
is numerically exact.

Per core, 2 batches of 4 images; SBUF partition layout [128 = (4 img, 32 ch)].
The three convs are FUSED per strip of R output rows with halo recompute
(strip s computes R+4 rows of h0 -> R+2 rows of h1 -> R output rows), so h0
and h1 never leave SBUF; only the padded sign(x) plane stages through DRAM.

 - conv0 (1->32ch): all 9 taps packed into K: input replicated into 36
   partitions (dy, dx, img) via 3 per-dy DMAs from the 3x column-shifted
   extended-pad sign(x) planes; one matmul per output row-pair
   (K=36, M=128, N=512).
 - conv1 (32->32ch): fp8 DoubleRow matmuls, 2 taps per pass: K=128 partitions
   x 2 k-subtiles; the rhs k-subtile offset is a free-dim shift on the padded
   input tile (4D AP), so 9 taps cost 5 passes instead of 9. Tap pairs share
   one (row,col) shift delta; the odd 9th tap is paired with zero weights.
 - conv2 (32->1ch): same DoubleRow pairing, M=4 (one column per image),
   psum [4, 4, 256] -> bf16 output (conv2 sums are even integers <= 288,
   exact in bf16).
PSUM is used as [128, 4, 256] 2-bank tiles, 4 in flight (2 row-pair matmul
groups + one batched eviction each). sign() evictions alternate between
ScalarE (activation Sign) and VectorE (clamp(-1,1) == sign for the integer
sums) so neither engine gates the psum rotation. Issue order is
software-pipelined globally over both batches at psum-tile granularity
(A of strip s, B of strip s-1, C of strip s-2 interleaved) so the in-order
PE queue never waits on an eviction tail.
"""

import numpy as np
import ml_dtypes

import concourse.bass as bass
import concourse.mybir as mybir
import concourse.tile as tile
from concourse import bacc
from concourse.bass_utils import run_bass_kernel_spmd

FP8 = mybir.dt.float8e4
BF16 = mybir.dt.bfloat16
F32 = mybir.dt.float32
AF = mybir.ActivationFunctionType
ALU = mybir.AluOpType
DRM = mybir.MatmulPerfMode.DoubleRow

N_CORES = 8
IMG_PER_CORE = 8
B = 4          # images per partition-batch
H = W = 256
WP = 258       # padded width (1 col pad each side)
HE = 262       # extended padded height: row = x row + 3
R = 64         # strip rows (output rows per strip)
NS = H // R    # strips per batch
NB = IMG_PER_CORE // B  # batches per core

# DoubleRow tap pairs: both taps of a pair share one flat shift delta
# (dy*WP + dx); the 9th tap is paired with zero weights (k-slot 1 unused).
PAIRS = [
    ((0, 0), (0, 1)),
    ((1, 0), (1, 1)),
    ((2, 0), (2, 1)),
    ((0, 2), (1, 2)),
    ((2, 2), None),
]


def _dr_rhs(hin, r, pair):
    """4D DoubleRow rhs AP: [128, ksub=2, rows=2, cols=256] with the ksub
    dim stepping by the tap-pair's shift delta over the padded tile."""
    (dy0, dx0), t1 = pair
    # the zero-weight dummy slot points one row up: always inside the tile
    delta = -WP if t1 is None else (t1[0] - dy0) * WP + (t1[1] - dx0)
    sl = hin[:, r + dy0:r + dy0 + 2, dx0:dx0 + 256]
    return bass.AP(
        tensor=sl.tensor, offset=sl.offset,
        ap=[list(sl.ap[0]), [delta, 2], list(sl.ap[1]), list(sl.ap[2])])


def _build_program(stages=('0', 'A', 'B', 'C')):
    nc = bacc.Bacc("TRN2", target_bir_lowering=False, debug=False)

    x_in = nc.dram_tensor("x", [IMG_PER_CORE, H, W], F32, kind="ExternalInput")
    s0_in = nc.dram_tensor("s0", [36, 128], FP8, kind="ExternalInput")
    s1_in = nc.dram_tensor("s1", [128, 5, 2, 128], FP8, kind="ExternalInput")
    s2_in = nc.dram_tensor("s2", [128, 5, 2, 16], FP8, kind="ExternalInput")
    out_d = nc.dram_tensor("out", [IMG_PER_CORE, H, W], BF16,
                           kind="ExternalOutput")

    # extended-pad sign(x), 3 column-shifted copies (one per conv dx tap):
    # xs3[b, dx, g, r, c] = sign(x)[img b*B+g, row r-3, col c+dx-1] with
    # zero padding outside; row = x row + 3 (rows 0-2 and 259-261 zero)
    xs3_d = nc.dram_tensor("xs3", [NB, 3, B, HE, 256], FP8)

    with tile.TileContext(nc) as tc:
        with (
            tc.tile_pool(name="const", bufs=1) as cpool,
            tc.tile_pool(name="xprep", bufs=4) as xpool,
            tc.tile_pool(name="xrep", bufs=2) as xrpool,
            tc.tile_pool(name="h0", bufs=2) as h0pool,
            tc.tile_pool(name="h1", bufs=2) as h1pool,
            tc.tile_pool(name="cout", bufs=2) as cpool2,
            tc.tile_pool(name="psum", bufs=4, space="PSUM") as pspool,
        ):
            # --- constants: stationary weights + a zero tile ---
            s0t = cpool.tile([36, 128], FP8, tag="s0")
            nc.sync.dma_start(out=s0t[:, :], in_=s0_in[:, :])
            s1t = cpool.tile([128, 5, 2, 128], FP8, tag="s1")
            nc.sync.dma_start(out=s1t[:, :, :, :], in_=s1_in[:, :, :, :])
            s2t = cpool.tile([128, 5, 2, 16], FP8, tag="s2")
            nc.sync.dma_start(out=s2t[:, :, :, :], in_=s2_in[:, :, :, :])
            zt = cpool.tile([128, 3 * 256], FP8, tag="zt")
            nc.gpsimd.memset(zt[:, :], 0.0)

            # --- pre-zero xs3 pad rows (cols baked into the stores) ---
            for b in range(NB):
                for r0, r1 in ((0, 3), (HE - 3, HE)):
                    nc.scalar.dma_start(
                        out=xs3_d[b, :, :, r0:r1, :].rearrange(
                            "a g r c -> (a g) r c"),
                        in_=zt[0:12, :].rearrange("p (r c) -> p r c", r=3))

            def stage_0(b):
                """sign(x) -> extended-pad fp8 planes in DRAM, batch b.
                One load + one sign for all 4 images (x rows in partitions,
                (img, 128-row block) merged in the free dim), then 12 shifted
                stores spread over the three DMA queues."""
                for g in range(B):
                    img = b * B + g
                    xf = xpool.tile([128, 2, W], F32, tag="xf")
                    nc.scalar.dma_start(
                        out=xf[:, :, :],
                        in_=bass.AP(tensor=x_in, offset=img * H * W,
                                    ap=[[W, 128], [128 * W, 2], [1, W]]))
                    xp = xpool.tile([128, 2, WP], FP8, tag="xp")
                    nc.scalar.activation(xp[:, :, 1:W + 1], xf[:, :, :],
                                         AF.Sign)
                    nc.gpsimd.memset(xp[:, :, 0:1], 0.0)
                    nc.gpsimd.memset(xp[:, :, WP - 1:WP], 0.0)
                    for dx in range(3):
                        nc.gpsimd.dma_start(
                            out=bass.AP(
                                tensor=xs3_d,
                                offset=(((b * 3 + dx) * B + g) * HE + 3) * 256,
                                ap=[[256, 128], [128 * 256, 2], [1, 256]]),
                            in_=xp[:, :, dx:dx + 256])

            if '0' in stages:
                stage_0(0)

            if True:
                ht0s, ht1s = {}, {}

                def stage_a_units(s, b):
                    """conv0 strip s: h0 rows [sR-2, sR+R+2) -> ht0 tile
                    (tile row i = h0 row sR-2+i). Returns (ht0, units)."""
                    xt = xrpool.tile([36, R + 4, 256], FP8, tag="xrep",
                                     name="xt")
                    for dy, q in ((0, nc.sync), (1, nc.sync),
                                  (2, nc.sync)):
                        q.dma_start(
                            out=xt[12 * dy:12 * dy + 12, :, :],
                            in_=xs3_d[b, :, :,
                                      s * R + dy:s * R + dy + R + 4, :]
                            .rearrange("a g r c -> (a g) r c"))
                    ht0 = h0pool.tile([128, R + 4, WP], FP8, tag="h0",
                                      name="ht0")
                    nc.gpsimd.memset(ht0[:, :, 0:1], 0.0)
                    nc.gpsimd.memset(ht0[:, :, WP - 1:WP], 0.0)

                    def unit(r0, last):
                        nrow = min(4, R + 4 - r0)
                        ps = pspool.tile([128, 4, 256], F32, tag="ps",
                                         name="psA")
                        for q in range(nrow // 2):
                            nc.tensor.matmul(
                                ps[:, 2 * q:2 * q + 2, :], s0t[:, :],
                                xt[:, r0 + 2 * q:r0 + 2 * q + 2, :],
                                start=True, stop=True)
                        if (r0 // 4) % 2 == 0:
                            nc.scalar.activation(
                                ht0[:, r0:r0 + nrow, 1:W + 1],
                                ps[:, 0:nrow, :], AF.Sign)
                        else:
                            nc.vector.tensor_scalar(
                                ht0[:, r0:r0 + nrow, 1:W + 1],
                                ps[:, 0:nrow, :], -1.0, 1.0,
                                ALU.max, ALU.min)
                        if last:
                            # boundary: h0 pad rows (-1 / 256) must be zero
                            if s == 0:
                                nc.gpsimd.memset(ht0[:, 1:2, :], 0.0)
                            if s == NS - 1:
                                nc.gpsimd.memset(ht0[:, R + 2:R + 3, :], 0.0)

                    r0s = list(range(0, R + 4, 4))
                    return ht0, [
                        (lambda r0=r0, last=(r0 == r0s[-1]): unit(r0, last))
                        for r0 in r0s]

                def stage_b_units(s, ht0):
                    """conv1 strip s: h1 rows [sR-1, sR+R+1) -> ht1 tile
                    (tile row i = h1 row sR-1+i); input ht0."""
                    ht1 = h1pool.tile([128, R + 2, WP], FP8, tag="h1",
                                      name="ht1")
                    nc.gpsimd.memset(ht1[:, :, 0:1], 0.0)
                    nc.gpsimd.memset(ht1[:, :, WP - 1:WP], 0.0)

                    def unit(r0, last):
                        nrow = min(4, R + 2 - r0)
                        ps = pspool.tile([128, 4, 256], F32, tag="ps",
                                         name="psB")
                        for q in range(nrow // 2):
                            for t, pair in enumerate(PAIRS):
                                nc.tensor.matmul(
                                    ps[:, 2 * q:2 * q + 2, :], s1t[:, t, :, :],
                                    _dr_rhs(ht0, r0 + 2 * q, pair),
                                    start=(t == 0), stop=(t == 4),
                                    perf_mode=DRM)
                        # alternate eviction engines to keep the psum
                        # rotation fed (clamp(-1,1) == sign for int sums)
                        if (r0 // 4) % 2 == 0:
                            nc.scalar.activation(
                                ht1[:, r0:r0 + nrow, 1:W + 1],
                                ps[:, 0:nrow, :], AF.Sign)
                        else:
                            nc.vector.tensor_scalar(
                                ht1[:, r0:r0 + nrow, 1:W + 1],
                                ps[:, 0:nrow, :], -1.0, 1.0,
                                ALU.max, ALU.min)
                        if last:
                            # boundary: h1 pad rows (-1 / 256) must be zero
                            if s == 0:
                                nc.gpsimd.memset(ht1[:, 0:1, :], 0.0)
                            if s == NS - 1:
                                nc.gpsimd.memset(ht1[:, R + 1:R + 2, :], 0.0)

                    r0s = list(range(0, R + 2, 4))
                    return ht1, [
                        (lambda r0=r0, last=(r0 == r0s[-1]): unit(r0, last))
                        for r0 in r0s]

                def stage_c_units(s, ht1, b):
                    """conv2 strip s: out rows [sR, sR+R); input ht1."""
                    ot = cpool2.tile([B, R, W], BF16, tag="c_out", name="ot")

                    def unit(r0, last):
                        ps = pspool.tile([128, 4, 256], F32, tag="ps",
                                         name="psC")
                        for q in range(2):
                            for t, pair in enumerate(PAIRS):
                                nc.tensor.matmul(
                                    ps[0:B, 2 * q:2 * q + 2, :],
                                    s2t[:, t, :, 0:B],
                                    _dr_rhs(ht1, r0 + 2 * q, pair),
                                    start=(t == 0), stop=(t == 4),
                                    perf_mode=DRM)
                        if (r0 // 4) % 2 == 0:
                            nc.vector.tensor_copy(
                                ot[:, r0:r0 + 4, :], ps[0:B, :, :])
                        else:
                            nc.scalar.activation(
                                ot[:, r0:r0 + 4, :], ps[0:B, :, :], AF.Copy)
                        if last:
                            nc.gpsimd.dma_start(
                                out=out_d[b * B:(b + 1) * B,
                                          s * R:s * R + R, :],
                                in_=ot[:, :, :])

                    r0s = list(range(0, R, 4))
                    return [
                        (lambda r0=r0, last=(r0 == r0s[-1]): unit(r0, last))
                        for r0 in r0s]

                # software pipeline, depth 2, interleaved at supertile
                # granularity and run GLOBALLY over both batches: fast-matmul
                # A units ride alongside slow-matmul B/C units so the
                # in-order PE never drains the 4-buffer PSUM rotation
                # waiting on an eviction.
                strips = [(b, s) for b in range(NB) for s in range(NS)]
                for gi in range(len(strips) + 2):
                    units = []
                    if gi < len(strips) and 'A' in stages:
                        gb, gs = strips[gi]
                        # next batch's input prep ahead of its first strip
                        if '0' in stages and gs == 0 and gb + 1 < NB:
                            stage_0(gb + 1)
                        ht0s[gi], ua = stage_a_units(gs, gb)
                        units.append(ua)
                    if 0 <= gi - 1 < len(strips) and 'B' in stages \
                            and gi - 1 in ht0s:
                        gb, gs = strips[gi - 1]
                        ht1s[gi - 1], ub = stage_b_units(gs,
                                                         ht0s.pop(gi - 1))
                        units.append(ub)
                    if 0 <= gi - 2 < len(strips) and 'C' in stages \
                            and gi - 2 in ht1s:
                        gb, gs = strips[gi - 2]
                        units.append(stage_c_units(gs, ht1s.pop(gi - 2), gb))
                    for i in range(max(map(len, units), default=0)):
                        for u in units:
                            if i < len(u):
                                u[i]()
    nc.compile()
    return nc


def _host_weights(w0, w1, w2):
    """Pack sign(w) into fp8 stationary matrices (see module docstring)."""
    f8 = ml_dtypes.float8_e4m3
    sg = lambda w: np.sign(np.asarray(w, np.float32))
    w0s, w1s, w2s = sg(w0), sg(w1), sg(w2)  # [32,1,3,3],[32,32,3,3],[1,32,3,3]
    s0 = np.zeros((36, 128), np.float32)
    s1 = np.zeros((128, 5, 2, 128), np.float32)
    s2 = np.zeros((128, 5, 2, 16), np.float32)
    for g in range(B):
        for dy in range(3):
            for dx in range(3):
                s0[(dy * 3 + dx) * 4 + g, g * 32:(g + 1) * 32] = \
                    w0s[:, 0, dy, dx]
        for t, ((dy0, dx0), t1) in enumerate(PAIRS):
            s1[g * 32:(g + 1) * 32, t, 0, g * 32:(g + 1) * 32] = \
                w1s[:, :, dy0, dx0].T  # [ci, co]
            s2[g * 32:(g + 1) * 32, t, 0, g] = w2s[0, :, dy0, dx0]
            if t1 is not None:
                s1[g * 32:(g + 1) * 32, t, 1, g * 32:(g + 1) * 32] = \
                    w1s[:, :, t1[0], t1[1]].T
                s2[g * 32:(g + 1) * 32, t, 1, g] = w2s[0, :, t1[0], t1[1]]
    return s0.astype(f8), s1.astype(f8), s2.astype(f8)


_NC_CACHE = {}


def kernel(x, w0, w1, w2):
    if "nc" not in _NC_CACHE:
        _NC_CACHE["nc"] = _build_program()
    nc = _NC_CACHE["nc"]
    s0, s1, s2 = _host_weights(w0, w1, w2)
    x = np.asarray(x, np.float32).reshape(64, H, W)
    in_maps = [
        {"x": np.ascontiguousarray(x[i * IMG_PER_CORE:(i + 1) * IMG_PER_CORE]),
         "s0": s0, "s1": s1, "s2": s2}
        for i in range(N_CORES)
    ]
    res = run_bass_kernel_spmd(nc, in_maps, list(range(N_CORES)))
    out = np.stack([np.asarray(res.results[i]["out"], np.float32)
                    for i in range(N_CORES)])
    return out.reshape(64, 1, H, W)


# revision 27
# speedup vs baseline: 1.3694x; 1.1102x over previous
"""Binary 3-layer CNN (sign activations + sign weights) on 8 NeuronCores.

Strategy: pure data parallel — 64 images -> 8 cores x 8 images, fp8 compute.
All matmul operands are exactly +-1/0 -> fp8e4m3 with fp32 PSUM accumulation
is numerically exact.

Per core, 2 batches of 4 images; SBUF partition layout [128 = (4 img, 32 ch)].
The three convs are FUSED per strip of R output rows with halo recompute
(strip s computes R+4 rows of h0 -> R+2 rows of h1 -> R output rows), so h0
and h1 never leave SBUF; only the padded sign(x) plane stages through DRAM.

 - conv0 (1->32ch): all 9 taps packed into K: input replicated into 36
   partitions (dy, dx, img) via 3 per-dy DMAs from the 3x column-shifted
   extended-pad sign(x) planes; one matmul per output row-pair
   (K=36, M=128, N=512).
 - conv1 (32->32ch): fp8 DoubleRow matmuls, 2 taps per pass: K=128 partitions
   x 2 k-subtiles; the rhs k-subtile offset is a free-dim shift on the padded
   input tile (4D AP), so 9 taps cost 5 passes instead of 9. Tap pairs share
   one (row,col) shift delta; the odd 9th tap is paired with zero weights.
 - conv2 (32->1ch): same DoubleRow pairing, M=4 (one column per image),
   psum [4, 4, 256] -> bf16 output (conv2 sums are even integers <= 288,
   exact in bf16).
PSUM is used as [128, 4, 256] 2-bank tiles, 4 in flight (2 row-pair matmul
groups + one batched eviction each). sign() evictions alternate between
ScalarE (activation Sign) and VectorE (clamp(-1,1) == sign for the integer
sums) so neither engine gates the psum rotation. Issue order is
software-pipelined globally over both batches at psum-tile granularity
(A of strip s, B of strip s-1, C of strip s-2 interleaved) so the in-order
PE queue never waits on an eviction tail.
"""

import numpy as np
import ml_dtypes

import concourse.bass as bass
import concourse.mybir as mybir
import concourse.tile as tile
from concourse import bacc
from concourse.bass_utils import run_bass_kernel_spmd

FP8 = mybir.dt.float8e4
BF16 = mybir.dt.bfloat16
F32 = mybir.dt.float32
AF = mybir.ActivationFunctionType
ALU = mybir.AluOpType
DRM = mybir.MatmulPerfMode.DoubleRow

N_CORES = 8
IMG_PER_CORE = 8
B = 4          # images per partition-batch
H = W = 256
WP = 258       # padded width (1 col pad each side)
HE = 262       # extended padded height: row = x row + 3
R = 64         # strip rows (output rows per strip)
NS = H // R    # strips per batch
NB = IMG_PER_CORE // B  # batches per core

# DoubleRow tap pairs: both taps of a pair share one flat shift delta
# (dy*WP + dx); the 9th tap is paired with zero weights (k-slot 1 unused).
PAIRS = [
    ((0, 0), (0, 1)),
    ((1, 0), (1, 1)),
    ((2, 0), (2, 1)),
    ((0, 2), (1, 2)),
    ((2, 2), None),
]

# (pair idx, k slot, dy, dx) of each real tap — the non-DoubleRow path
TAPS9 = [(t, k, *tap) for t, pr in enumerate(PAIRS)
         for k, tap in enumerate(pr) if tap is not None]


def _dr_rhs(hin, r, pair):
    """4D DoubleRow rhs AP: [128, ksub=2, rows=2, cols=256] with the ksub
    dim stepping by the tap-pair's shift delta over the padded tile."""
    (dy0, dx0), t1 = pair
    # the zero-weight dummy slot points one row up: always inside the tile
    delta = -WP if t1 is None else (t1[0] - dy0) * WP + (t1[1] - dx0)
    sl = hin[:, r + dy0:r + dy0 + 2, dx0:dx0 + 256]
    return bass.AP(
        tensor=sl.tensor, offset=sl.offset,
        ap=[list(sl.ap[0]), [delta, 2], list(sl.ap[1]), list(sl.ap[2])])


def _build_program(stages=('0', 'A', 'B', 'C'), use_dr=True):
    nc = bacc.Bacc("TRN2", target_bir_lowering=False, debug=False)

    x_in = nc.dram_tensor("x", [IMG_PER_CORE, H, W], F32, kind="ExternalInput")
    s0_in = nc.dram_tensor("s0", [36, 128], FP8, kind="ExternalInput")
    s1_in = nc.dram_tensor("s1", [128, 5, 2, 128], FP8, kind="ExternalInput")
    s2_in = nc.dram_tensor("s2", [128, 5, 2, 16], FP8, kind="ExternalInput")
    out_d = nc.dram_tensor("out", [IMG_PER_CORE, H, W], BF16,
                           kind="ExternalOutput")

    # extended-pad sign(x), 3 column-shifted copies (one per conv dx tap):
    # xs3[b, dx, g, r, c] = sign(x)[img b*B+g, row r-3, col c+dx-1] with
    # zero padding outside; row = x row + 3 (rows 0-2 and 259-261 zero)
    xs3_d = nc.dram_tensor("xs3", [NB, 3, B, HE, 256], FP8)

    with tile.TileContext(nc) as tc:
        with (
            tc.tile_pool(name="const", bufs=1) as cpool,
            tc.tile_pool(name="xprep", bufs=4) as xpool,
            tc.tile_pool(name="xrep", bufs=2) as xrpool,
            tc.tile_pool(name="h0", bufs=2) as h0pool,
            tc.tile_pool(name="h1", bufs=2) as h1pool,
            tc.tile_pool(name="cout", bufs=2) as cpool2,
            tc.tile_pool(name="psum", bufs=4, space="PSUM") as pspool,
        ):
            # --- constants: stationary weights + a zero tile ---
            s0t = cpool.tile([36, 128], FP8, tag="s0")
            nc.sync.dma_start(out=s0t[:, :], in_=s0_in[:, :])
            s1t = cpool.tile([128, 5, 2, 128], FP8, tag="s1")
            nc.sync.dma_start(out=s1t[:, :, :, :], in_=s1_in[:, :, :, :])
            s2t = cpool.tile([128, 5, 2, 16], FP8, tag="s2")
            nc.sync.dma_start(out=s2t[:, :, :, :], in_=s2_in[:, :, :, :])
            zt = cpool.tile([128, 3 * 256], FP8, tag="zt")
            nc.gpsimd.memset(zt[:, :], 0.0)

            # --- pre-zero xs3 pad rows (cols baked into the stores) ---
            for b in range(NB):
                for r0, r1 in ((0, 3), (HE - 3, HE)):
                    nc.scalar.dma_start(
                        out=xs3_d[b, :, :, r0:r1, :].rearrange(
                            "a g r c -> (a g) r c"),
                        in_=zt[0:12, :].rearrange("p (r c) -> p r c", r=3))

            def stage_0(b):
                """sign(x) -> extended-pad fp8 planes in DRAM, batch b.
                One load + one sign for all 4 images (x rows in partitions,
                (img, 128-row block) merged in the free dim), then 12 shifted
                stores spread over the three DMA queues."""
                for g in range(B):
                    img = b * B + g
                    xf = xpool.tile([128, 2, W], F32, tag="xf")
                    nc.scalar.dma_start(
                        out=xf[:, :, :],
                        in_=bass.AP(tensor=x_in, offset=img * H * W,
                                    ap=[[W, 128], [128 * W, 2], [1, W]]))
                    xp = xpool.tile([128, 2, WP], FP8, tag="xp")
                    nc.scalar.activation(xp[:, :, 1:W + 1], xf[:, :, :],
                                         AF.Sign)
                    nc.gpsimd.memset(xp[:, :, 0:1], 0.0)
                    nc.gpsimd.memset(xp[:, :, WP - 1:WP], 0.0)
                    for dx in range(3):
                        nc.gpsimd.dma_start(
                            out=bass.AP(
                                tensor=xs3_d,
                                offset=(((b * 3 + dx) * B + g) * HE + 3) * 256,
                                ap=[[256, 128], [128 * 256, 2], [1, 256]]),
                            in_=xp[:, :, dx:dx + 256])

            if '0' in stages:
                stage_0(0)

            if True:
                ht0s, ht1s = {}, {}

                def stage_a_units(s, b):
                    """conv0 strip s: h0 rows [sR-2, sR+R+2) -> ht0 tile
                    (tile row i = h0 row sR-2+i). Returns (ht0, units)."""
                    xt = xrpool.tile([36, R + 4, 256], FP8, tag="xrep",
                                     name="xt")
                    for dy, q in ((0, nc.sync), (1, nc.sync),
                                  (2, nc.sync)):
                        q.dma_start(
                            out=xt[12 * dy:12 * dy + 12, :, :],
                            in_=xs3_d[b, :, :,
                                      s * R + dy:s * R + dy + R + 4, :]
                            .rearrange("a g r c -> (a g) r c"))
                    ht0 = h0pool.tile([128, R + 4, WP], FP8, tag="h0",
                                      name="ht0")
                    nc.gpsimd.memset(ht0[:, :, 0:1], 0.0)
                    nc.gpsimd.memset(ht0[:, :, WP - 1:WP], 0.0)

                    def unit(r0, last):
                        nrow = min(4, R + 4 - r0)
                        ps = pspool.tile([128, 4, 256], F32, tag="ps",
                                         name="psA")
                        for q in range(nrow // 2):
                            nc.tensor.matmul(
                                ps[:, 2 * q:2 * q + 2, :], s0t[:, :],
                                xt[:, r0 + 2 * q:r0 + 2 * q + 2, :],
                                start=True, stop=True)
                        if (r0 // 4) % 2 == 0:
                            nc.scalar.activation(
                                ht0[:, r0:r0 + nrow, 1:W + 1],
                                ps[:, 0:nrow, :], AF.Sign)
                        else:
                            nc.vector.tensor_scalar(
                                ht0[:, r0:r0 + nrow, 1:W + 1],
                                ps[:, 0:nrow, :], -1.0, 1.0,
                                ALU.max, ALU.min)
                        if last:
                            # boundary: h0 pad rows (-1 / 256) must be zero
                            if s == 0:
                                nc.gpsimd.memset(ht0[:, 1:2, :], 0.0)
                            if s == NS - 1:
                                nc.gpsimd.memset(ht0[:, R + 2:R + 3, :], 0.0)

                    r0s = list(range(0, R + 4, 4))
                    return ht0, [
                        (lambda r0=r0, last=(r0 == r0s[-1]): unit(r0, last))
                        for r0 in r0s]

                def stage_b_units(s, ht0):
                    """conv1 strip s: h1 rows [sR-1, sR+R+1) -> ht1 tile
                    (tile row i = h1 row sR-1+i); input ht0."""
                    ht1 = h1pool.tile([128, R + 2, WP], FP8, tag="h1",
                                      name="ht1")
                    nc.gpsimd.memset(ht1[:, :, 0:1], 0.0)
                    nc.gpsimd.memset(ht1[:, :, WP - 1:WP], 0.0)

                    def unit(r0, last):
                        nrow = min(4, R + 2 - r0)
                        ps = pspool.tile([128, 4, 256], F32, tag="ps",
                                         name="psB")
                        for q in range(nrow // 2):
                            if use_dr:
                                for t, pair in enumerate(PAIRS):
                                    nc.tensor.matmul(
                                        ps[:, 2 * q:2 * q + 2, :],
                                        s1t[:, t, :, :],
                                        _dr_rhs(ht0, r0 + 2 * q, pair),
                                        start=(t == 0), stop=(t == 4),
                                        perf_mode=DRM)
                            else:
                                for ti, (t, k, dy, dx) in enumerate(TAPS9):
                                    nc.tensor.matmul(
                                        ps[:, 2 * q:2 * q + 2, :],
                                        s1t[:, t, k, :],
                                        ht0[:, r0 + 2 * q + dy:
                                            r0 + 2 * q + dy + 2,
                                            dx:dx + 256],
                                        start=(ti == 0), stop=(ti == 8))
                        # alternate eviction engines to keep the psum
                        # rotation fed (clamp(-1,1) == sign for int sums)
                        if (r0 // 4) % 2 == 0:
                            nc.scalar.activation(
                                ht1[:, r0:r0 + nrow, 1:W + 1],
                                ps[:, 0:nrow, :], AF.Sign)
                        else:
                            nc.vector.tensor_scalar(
                                ht1[:, r0:r0 + nrow, 1:W + 1],
                                ps[:, 0:nrow, :], -1.0, 1.0,
                                ALU.max, ALU.min)
                        if last:
                            # boundary: h1 pad rows (-1 / 256) must be zero
                            if s == 0:
                                nc.gpsimd.memset(ht1[:, 0:1, :], 0.0)
                            if s == NS - 1:
                                nc.gpsimd.memset(ht1[:, R + 1:R + 2, :], 0.0)

                    r0s = list(range(0, R + 2, 4))
                    return ht1, [
                        (lambda r0=r0, last=(r0 == r0s[-1]): unit(r0, last))
                        for r0 in r0s]

                def stage_c_units(s, ht1, b):
                    """conv2 strip s: out rows [sR, sR+R); input ht1."""
                    ot = cpool2.tile([B, R, W], BF16, tag="c_out", name="ot")

                    def unit(r0, last):
                        ps = pspool.tile([128, 4, 256], F32, tag="ps",
                                         name="psC")
                        for q in range(2):
                            if use_dr:
                                for t, pair in enumerate(PAIRS):
                                    nc.tensor.matmul(
                                        ps[0:B, 2 * q:2 * q + 2, :],
                                        s2t[:, t, :, 0:B],
                                        _dr_rhs(ht1, r0 + 2 * q, pair),
                                        start=(t == 0), stop=(t == 4),
                                        perf_mode=DRM)
                            else:
                                for ti, (t, k, dy, dx) in enumerate(TAPS9):
                                    nc.tensor.matmul(
                                        ps[0:B, 2 * q:2 * q + 2, :],
                                        s2t[:, t, k, 0:B],
                                        ht1[:, r0 + 2 * q + dy:
                                            r0 + 2 * q + dy + 2,
                                            dx:dx + 256],
                                        start=(ti == 0), stop=(ti == 8))
                        if (r0 // 4) % 2 == 0:
                            nc.vector.tensor_copy(
                                ot[:, r0:r0 + 4, :], ps[0:B, :, :])
                        else:
                            nc.scalar.activation(
                                ot[:, r0:r0 + 4, :], ps[0:B, :, :], AF.Copy)
                        if last:
                            nc.gpsimd.dma_start(
                                out=out_d[b * B:(b + 1) * B,
                                          s * R:s * R + R, :],
                                in_=ot[:, :, :])

                    r0s = list(range(0, R, 4))
                    return [
                        (lambda r0=r0, last=(r0 == r0s[-1]): unit(r0, last))
                        for r0 in r0s]

                # software pipeline, depth 2, interleaved at supertile
                # granularity and run GLOBALLY over both batches: fast-matmul
                # A units ride alongside slow-matmul B/C units so the
                # in-order PE never drains the 4-buffer PSUM rotation
                # waiting on an eviction.
                strips = [(b, s) for b in range(NB) for s in range(NS)]
                for gi in range(len(strips) + 2):
                    units = []
                    if gi < len(strips) and 'A' in stages:
                        gb, gs = strips[gi]
                        # next batch's input prep ahead of its first strip
                        if '0' in stages and gs == 0 and gb + 1 < NB:
                            stage_0(gb + 1)
                        ht0s[gi], ua = stage_a_units(gs, gb)
                        units.append(ua)
                    if 0 <= gi - 1 < len(strips) and 'B' in stages \
                            and gi - 1 in ht0s:
                        gb, gs = strips[gi - 1]
                        ht1s[gi - 1], ub = stage_b_units(gs,
                                                         ht0s.pop(gi - 1))
                        units.append(ub)
                    if 0 <= gi - 2 < len(strips) and 'C' in stages \
                            and gi - 2 in ht1s:
                        gb, gs = strips[gi - 2]
                        units.append(stage_c_units(gs, ht1s.pop(gi - 2), gb))
                    for i in range(max(map(len, units), default=0)):
                        for u in units:
                            if i < len(u):
                                u[i]()
    nc.compile()
    return nc


def _host_weights(w0, w1, w2):
    """Pack sign(w) into fp8 stationary matrices (see module docstring)."""
    f8 = ml_dtypes.float8_e4m3
    sg = lambda w: np.sign(np.asarray(w, np.float32))
    w0s, w1s, w2s = sg(w0), sg(w1), sg(w2)  # [32,1,3,3],[32,32,3,3],[1,32,3,3]
    s0 = np.zeros((36, 128), np.float32)
    s1 = np.zeros((128, 5, 2, 128), np.float32)
    s2 = np.zeros((128, 5, 2, 16), np.float32)
    for g in range(B):
        for dy in range(3):
            for dx in range(3):
                s0[(dy * 3 + dx) * 4 + g, g * 32:(g + 1) * 32] = \
                    w0s[:, 0, dy, dx]
        for t, ((dy0, dx0), t1) in enumerate(PAIRS):
            s1[g * 32:(g + 1) * 32, t, 0, g * 32:(g + 1) * 32] = \
                w1s[:, :, dy0, dx0].T  # [ci, co]
            s2[g * 32:(g + 1) * 32, t, 0, g] = w2s[0, :, dy0, dx0]
            if t1 is not None:
                s1[g * 32:(g + 1) * 32, t, 1, g * 32:(g + 1) * 32] = \
                    w1s[:, :, t1[0], t1[1]].T
                s2[g * 32:(g + 1) * 32, t, 1, g] = w2s[0, :, t1[0], t1[1]]
    return s0.astype(f8), s1.astype(f8), s2.astype(f8)


_NC_CACHE = {}


def kernel(x, w0, w1, w2):
    if "nc" not in _NC_CACHE:
        _NC_CACHE["nc"] = _build_program()
    nc = _NC_CACHE["nc"]
    s0, s1, s2 = _host_weights(w0, w1, w2)
    x = np.asarray(x, np.float32).reshape(64, H, W)
    in_maps = [
        {"x": np.ascontiguousarray(x[i * IMG_PER_CORE:(i + 1) * IMG_PER_CORE]),
         "s0": s0, "s1": s1, "s2": s2}
        for i in range(N_CORES)
    ]
    res = run_bass_kernel_spmd(nc, in_maps, list(range(N_CORES)))
    out = np.stack([np.asarray(res.results[i]["out"], np.float32)
                    for i in range(N_CORES)])
    return out.reshape(64, 1, H, W)
